# revision 22
# baseline (speedup 1.0000x reference)
"""Multi-head attention TRN2 Bass kernel (v2).

Sharding: head-parallel across 8 cores (2 heads each). Each core computes
its heads' contribution through the row-sharded W_O matmul; the host sums
the 8 partial (N, D_MODEL) outputs (f16) and adds the bias terms.

Per-core dataflow (all matmul inputs bf16, fp32 PSUM accumulation):
  KhT (128 x M)  = [Wk_h0|Wk_h1].T @ K.T   (head h in partitions 64h..64h+63)
  QhT (128 x N)  = same for Q
  Vh  (m x 128)  = V.T_tile.T @ Wv          (data-stationary; both heads side
                                             by side, no PE transpose needed)
  per n-chunk of 512, per m-tile pair:
    ST_h (m x n) = KhT_h[:, mtile].T @ QhT_h[:, chunk]  -- the two heads run
                   CONCURRENTLY in the PE array via tile_position row packing
                   (contract dim is only d_k=64)
    E_h = exp(ST_h - 20)  one scalar-engine activation per (head, mt-PAIR)
                   reading (128, 1024) from 2 PSUM banks to amortize the
                   ~352-cycle per-instruction ACT overhead
  per n-tile of 128 (after all E of the chunk):
    U_h (n x 65) += E_h_tile.T @ [Vh_h | ones]   (E is the stationary operand;
                   col 64 accumulates the softmax denominator for free)
    G   (n x 128) = U[:, :64] * recip(U[:, 64])  (per-partition scalar mul)
    GT  (128 x n) = PE-transpose(G)
    partial (n x D_MODEL) = GT.T @ (dk^-0.5 * Wo_c)  -> f16 -> DRAM

Host: out = sum_c partial_c + dk^-0.5 * (bv_flat @ Wo) + bo
"""

import os
from contextlib import ExitStack

import ml_dtypes
import numpy as np

N, M, D_MODEL, H, D_K, D_V = 2048, 2048, 1024, 16, 64, 64
NCORES = 8
HPC = H // NCORES        # heads per core = 2
DH = HPC * D_K           # 128 = stacked head dim per core
CH = 512                 # n-chunk (matmul moving free size)
NCH = N // CH            # 4
KT = D_MODEL // 128      # 8 contraction tiles for projections
MT = M // 128            # 16 m tiles
W2 = 2 * (D_V + 1)       # 130: per-mt vh_aug block: [Vh0|1|Vh1|1]
EXP_BIAS = -20.0         # constant shift, cancels in softmax; guards overflow

_compiled = {}
LAST_RESULT = {}


def _build_bass():
    import concourse.tile as tile
    from concourse import bacc, mybir
    from concourse.masks import make_identity

    f32 = mybir.dt.float32
    f16 = mybir.dt.float16
    bf16 = mybir.dt.bfloat16
    nc = bacc.Bacc(
        "TRN2",
        target_bir_lowering=False,
        debug=False,
        enable_asserts=False,
        num_devices=NCORES,
    )

    qt = nc.dram_tensor("qt", (D_MODEL, N), bf16, kind="ExternalInput").ap()
    kti = nc.dram_tensor("kt", (D_MODEL, M), bf16, kind="ExternalInput").ap()
    vti = nc.dram_tensor("vt", (D_MODEL, M), bf16, kind="ExternalInput").ap()
    # host pre-swizzles projection weights into SBUF layout (128, KT*DH)
    wq = nc.dram_tensor("wq", (128, KT * DH), bf16, kind="ExternalInput").ap()
    wk = nc.dram_tensor("wk", (128, KT * DH), bf16, kind="ExternalInput").ap()
    wv = nc.dram_tensor("wv", (128, KT * DH), bf16, kind="ExternalInput").ap()
    wo = nc.dram_tensor("wo", (DH, D_MODEL), bf16, kind="ExternalInput").ap()
    bq = nc.dram_tensor("bq", (DH, 1), f32, kind="ExternalInput").ap()
    bk = nc.dram_tensor("bk", (DH, 1), f32, kind="ExternalInput").ap()
    out = nc.dram_tensor("out", (N, D_MODEL), f16, kind="ExternalOutput").ap()

    Exp = mybir.ActivationFunctionType.Exp

    with tile.TileContext(nc) as tc, ExitStack() as ctx:
        cpool = ctx.enter_context(tc.tile_pool(name="const", bufs=1))

        wq_sb = cpool.tile([128, D_MODEL], bf16, tag="wq")
        wk_sb = cpool.tile([128, D_MODEL], bf16, tag="wk")
        wv_sb = cpool.tile([128, D_MODEL], bf16, tag="wv")
        wo_sb = cpool.tile([128, D_MODEL], bf16, tag="wo")
        bq_sb = cpool.tile([DH, 1], f32, tag="bq")
        bk_sb = cpool.tile([DH, 1], f32, tag="bk")
        id_sb = cpool.tile([128, 128], bf16, tag="id")
        eb_sb = cpool.tile([128, 1], f32, tag="eb")
        qht = cpool.tile([DH, N], bf16, tag="qht")
        kht = cpool.tile([DH, M], bf16, tag="kht")
        vh_aug = cpool.tile([128, MT * W2], bf16, tag="vaug")
        # full transposed inputs staged in SBUF via 8 big DMAs each
        qts = cpool.tile([128, KT * N], bf16, tag="qts")
        kts = cpool.tile([128, KT * M], bf16, tag="kts")
        vts = cpool.tile([128, KT * M], bf16, tag="vts")

        # weights on the scalar DMA queue so they don't serialize behind
        # the activation streams on the sync queue
        nc.scalar.dma_start(wq_sb[:], wq[:, :])
        nc.scalar.dma_start(wk_sb[:], wk[:, :])
        nc.scalar.dma_start(wv_sb[:], wv[:, :])
        nc.scalar.dma_start(wo_sb[:], wo[:, :])
        nc.scalar.dma_start(bq_sb[:], bq[:, :])
        nc.scalar.dma_start(bk_sb[:], bk[:, :])
        make_identity(nc, id_sb[:])
        nc.gpsimd.memset(vh_aug[:], 1.0)
        nc.gpsimd.memset(eb_sb[:], EXP_BIAS)

        with tc.tile_pool(name="ps", bufs=1, space="PSUM") as pp, \
                tc.tile_pool(name="wk2", bufs=2) as wpool:

            # PE warm-up: the HAM clock gate defaults to 1.2GHz and needs
            # ~3.4us of sustained activity to release to 2.4GHz. Keep the PE
            # ticking through the DMA-bound ramp so projections run warm.
            warm = pp.tile([128, 2 * CH], f32, tag="st0", bufs=1, name="warm")
            for i in range(75):
                nc.tensor.matmul(warm[:, 0:128], id_sb[:], id_sb[:],
                                 start=True, stop=True, skip_group_check=True)

            # input DMAs, half-M chunks (2KB DRAM lines = full DMA rate),
            # ordered by when compute first needs them: scores for m-pairs
            # 0-3 need K half 0 + Q chunk 0-1 only.
            HM = M // 2
            # critical path: per-k descriptors so projections chase arrivals
            for k in range(KT):
                nc.sync.dma_start(kts[:, k * M:k * M + HM], kti[k * 128:(k + 1) * 128, 0:HM])
            for k in range(KT):
                nc.sync.dma_start(qts[:, k * N:k * N + HM], qt[k * 128:(k + 1) * 128, 0:HM])
            # latency-insensitive halves: one multi-dim descriptor per group
            # (the sync queue's ~700ns per-descriptor issue adds up fast)
            kts3 = kts.rearrange("p (k m) -> p k m", k=KT)
            qts3 = qts.rearrange("p (k m) -> p k m", k=KT)
            vts3 = vts.rearrange("p (k m) -> p k m", k=KT)
            kti3 = kti.rearrange("(k p) m -> p k m", p=128)
            qt3 = qt.rearrange("(k p) m -> p k m", p=128)
            vti3 = vti.rearrange("(k p) m -> p k m", p=128)
            nc.sync.dma_start(kts3[:, :, HM:M], kti3[:, :, HM:M])
            nc.sync.dma_start(vts3[:, :, 0:HM], vti3[:, :, 0:HM])
            nc.sync.dma_start(vts3[:, :, HM:M], vti3[:, :, HM:M])
            nc.sync.dma_start(qts3[:, :, HM:N], qt3[:, :, HM:N])

            def proj_pass(x_sb, w_sb, out_sb, bias_sb, chunks):
                # 2 chunks per pass, k-outer so matmuls chase the input DMAs
                ts = {}
                for ch in chunks:
                    ts[ch] = pp.tile([128, CH], f32, tag="sc", bufs=2, name=f"pj{ch}")
                for k in range(KT):
                    for ch in chunks:
                        nc.tensor.matmul(
                            ts[ch][:],
                            w_sb[:, k * DH:(k + 1) * DH],
                            x_sb[:, k * N + ch * CH:k * N + (ch + 1) * CH],
                            start=(k == 0),
                            stop=(k == KT - 1),
                        )
                for ch in chunks:
                    nc.vector.tensor_scalar_add(
                        out_sb[:, ch * CH:(ch + 1) * CH], ts[ch][:], bias_sb[:]
                    )

            def vproj_pass(g):
                # 4 m-tiles per pass, direct (m x dh) layout: data stationary.
                # k-inner: each sub-region's accumulation group is contiguous
                # (start=True clears has_written for the WHOLE bank, so
                # interleaved groups in one bank would corrupt accumulation)
                vp = pp.tile([128, CH], f32, tag="sc", bufs=2, name="vp")
                for i, mt in enumerate(g):
                    for k in range(KT):
                        nc.tensor.matmul(
                            vp[:, i * 128:(i + 1) * 128],
                            vts[:, k * M + mt * 128:k * M + (mt + 1) * 128],
                            wv_sb[:, k * DH:(k + 1) * DH],
                            start=(k == 0),
                            stop=(k == KT - 1),
                        )
                for i, mt in enumerate(g):
                    b = mt * W2
                    nc.vector.tensor_copy(vh_aug[:, b:b + D_V], vp[:, i * 128:i * 128 + D_V])
                    nc.vector.tensor_copy(
                        vh_aug[:, b + D_V + 1:b + W2 - 1],
                        vp[:, i * 128 + D_V:i * 128 + 2 * D_V],
                    )

            # only what the first score pairs need goes upfront; the rest of
            # the projections drain into chunk 0's exp-wait gaps
            proj_pass(kts, wk_sb, kht, bk_sb, [0, 1])
            proj_pass(qts, wq_sb, qht, bq_sb, [0, 1])

            def make_pv_emit(ex_pairs, u_state):
                # PV chain instructions for one mt-pair; chains for all
                # (h, nt) accumulate into 2 U banks (2 nt's + 2 heads per
                # bank). Only the globally-first matmul into each bank uses
                # start=True: start clears has_written for the WHOLE bank,
                # so per-chain starts would wipe sibling chains' state.
                def pv_emit(p):
                    if "A" not in u_state:
                        u_state["A"] = pp.tile([128, 4 * (D_V + 1)], f32,
                                               tag="u", bufs=2, name="uA")
                        u_state["B"] = pp.tile([128, 4 * (D_V + 1)], f32,
                                               tag="u", bufs=2, name="uB")
                        u_state["started"] = set()
                    for j in range(2):
                        mt = 2 * p + j
                        for h in range(HPC):
                            for nt in range(4):
                                key = "A" if nt < 2 else "B"
                                u = u_state[key]
                                off = (nt % 2) * 130 + h * 65
                                first = key not in u_state["started"]
                                u_state["started"].add(key)
                                last = (mt == MT - 1 and h == HPC - 1
                                        and nt % 2 == 1)
                                ex = ex_pairs[(h, p)]
                                eoff = j * CH + nt * 128
                                nc.tensor.matmul(
                                    u[:, off:off + 65],
                                    ex[:, eoff:eoff + 128],
                                    vh_aug[:, mt * W2 + h * 65:mt * W2 + h * 65 + 65],
                                    start=first,
                                    stop=last,
                                    skip_group_check=True,
                                )
                return pv_emit

            def build_tail(c, u_state):
                # post-chain per-chunk work: normalize, transpose, Wo, out.
                # Each item is (pe_cost, fn) so the drain loop can smooth PE
                # work across the next chunk's exp-wait gaps. Norms go first:
                # they free the U banks for the next chunk's chains.
                work = []
                g_tiles = {}

                def norm(nt):
                    def f():
                        u = u_state["A" if nt < 2 else "B"]
                        off = (nt % 2) * 130
                        g = wpool.tile([128, 128], bf16, tag="g", bufs=3, name=f"g{nt}")
                        g_tiles[nt] = g
                        for h in range(HPC):
                            rcp = wpool.tile([128, 1], f32, tag="rcp", bufs=4, name=f"rcp{nt}_{h}")
                            nc.vector.reciprocal(
                                rcp[:], u[:, off + h * 65 + D_V:off + h * 65 + D_V + 1])
                            nc.vector.tensor_scalar_mul(
                                g[:, h * D_V:(h + 1) * D_V],
                                u[:, off + h * 65:off + h * 65 + D_V], rcp[:]
                            )
                    return f

                gt_tiles = {}

                def gtrans(nt):
                    def f():
                        gp = pp.tile([128, 128], bf16, tag="sc", bufs=2, name=f"gp{nt}")
                        gt = wpool.tile([128, 128], bf16, tag="gt", bufs=3, name=f"gt{nt}")
                        gt_tiles[nt] = gt
                        nc.tensor.transpose(gp[:], g_tiles[nt][:], id_sb[:])
                        nc.vector.tensor_copy(gt[:], gp[:])
                    return f

                def wo_phase(nt):
                    def f():
                        n0 = c * CH + nt * 128
                        ob = wpool.tile([128, D_MODEL], f16, tag="ob", bufs=3, name=f"ob{nt}")
                        for half in range(2):
                            wp = pp.tile([128, CH], f32, tag="sc", bufs=2, name=f"wp{nt}_{half}")
                            nc.tensor.matmul(
                                wp[:],
                                gt_tiles[nt][:],
                                wo_sb[:, half * CH:(half + 1) * CH],
                                start=True,
                                stop=True,
                            )
                            nc.vector.tensor_copy(ob[:, half * CH:(half + 1) * CH], wp[:])
                        nc.sync.dma_start(out[n0:n0 + 128, :], ob[:])
                    return f

                for nt in range(4):
                    work.append((0.0, norm(nt)))
                for nt in range(4):
                    work.append((0.5, gtrans(nt)))
                    work.append((2.0, wo_phase(nt)))
                return work

            tail = []
            for c in range(NCH):
                # extra PE work to interleave into this chunk's exp-paced
                # score phase (runs in the gaps while ScalarE does exps)
                filler = list(tail)
                if c == 0:
                    # K proj m-half 1; drained at p2/p3 when its DMA lands
                    filler = [
                        (2.0, lambda: proj_pass(kts, wk_sb, kht, bk_sb, [2])),
                        (2.0, lambda: proj_pass(kts, wk_sb, kht, bk_sb, [3])),
                    ]
                    # V DMA lands mid-chunk0; schedule vproj passes late in
                    # the pair loop so they don't head-of-line-block the PE
                    vwork = [lambda g=g: vproj_pass(g)
                             for g in ([0, 1, 2, 3], [4, 5, 6, 7],
                                       [8, 9, 10, 11], [12, 13, 14, 15])]
                else:
                    vwork = []

                ex_pairs = {}
                u_state = {}
                pv_emit = make_pv_emit(ex_pairs, u_state)
                npairs = MT // 2
                for p in range(npairs):
                    st0 = pp.tile([128, 2 * CH], f32, tag="st0", bufs=1)
                    st1 = pp.tile([128, 2 * CH], f32, tag="st1", bufs=1)
                    for j in range(2):  # j: which mt of the pair
                        mt = 2 * p + j
                        nc.tensor.matmul(
                            st0[:, j * CH:(j + 1) * CH],
                            kht[0:64, mt * 128:(mt + 1) * 128],
                            qht[0:64, c * CH:(c + 1) * CH],
                            start=True, stop=True,
                            tile_position=(0, 0),
                        )
                        nc.tensor.matmul(
                            st1[:, j * CH:(j + 1) * CH],
                            kht[64:128, mt * 128:(mt + 1) * 128],
                            qht[64:128, c * CH:(c + 1) * CH],
                            start=True, stop=True,
                            tile_position=(64, 0),
                        )
                    ex0 = wpool.tile([128, 2 * CH], bf16, tag="ex", bufs=24)
                    ex1 = wpool.tile([128, 2 * CH], bf16, tag="ex", bufs=24)
                    nc.scalar.activation(ex0[:], st0[:], Exp, bias=eb_sb[:])
                    nc.scalar.activation(ex1[:], st1[:], Exp, bias=eb_sb[:])
                    ex_pairs[(0, p)] = ex0
                    ex_pairs[(1, p)] = ex1

                    if c == 0:
                        # chunk 0: K half 1 / V arrive mid-chunk, so the
                        # deferred work drains at fixed pair slots matched
                        # to the DMA schedule; PV bunches once V is ready
                        if p in (2, 3) and filler:
                            filler.pop(0)[1]()
                        if p >= 4 and vwork:
                            vwork.pop(0)()
                        if p == 6:
                            pv_emit(0)
                            pv_emit(1)
                        elif p == 7:
                            pv_emit(2)
                            pv_emit(3)
                    else:
                        # steady state: drain deferred tail work first (the
                        # previous chunk's chains+norms must finish before
                        # this chunk's first pv_emit reuses the U banks),
                        # then PV chases behind so it never sits in the PE
                        # queue ahead of the next ST (which gates the next
                        # exp); the last chunk chases tighter to cut the
                        # end-of-kernel tail
                        budget = 2.6
                        while budget > 0 and filler:
                            cost, f = filler.pop(0)
                            f()
                            budget -= cost
                        lag = 1 if c == NCH - 1 else 2
                        if p >= lag:
                            pv_emit(p - lag)
                if c == NCH - 1:
                    pv_emit(npairs - 1)
                while vwork:
                    vwork.pop(0)()
                while filler:
                    filler.pop(0)[1]()
                tail = build_tail(c, u_state)
                if c < NCH - 1:
                    # finish this chunk's last two PV pairs inside the next
                    # chunk's drain instead of bunching them here (a bunch
                    # ahead of the next chunk's STs stalls its first exps)
                    tail.insert(0, (1.3, lambda pe=pv_emit: pe(npairs - 2)))
                    tail.insert(1, (1.3, lambda pe=pv_emit: pe(npairs - 1)))
                if c == 0:
                    # chunk 0 extras, in dependency order: the PV pairs
                    # deferred while V was still loading, and Q proj chunks
                    # 2-3 (whose DMA lands last)
                    pre = [(1.3, lambda q=q, pe=pv_emit: pe(q)) for q in range(4, npairs - 2)]
                    tail = pre + tail
                    tail.insert(len(pre) + 2 + 4 + 4,
                                (2.0, lambda: proj_pass(qts, wq_sb, qht, bq_sb, [2])))
                    tail.insert(len(pre) + 2 + 4 + 5,
                                (2.0, lambda: proj_pass(qts, wq_sb, qht, bq_sb, [3])))
            # last chunk: execute the tail stage-ordered (norms; transposes
            # and Wo interleaved two-at-a-time across the two scratch banks)
            # so PE and DVE ping-pong instead of serializing per n-tile
            t = tail
            for idx in (0, 1, 2, 3, 4, 6, 5, 7, 8, 10, 9, 11):
                t[idx][1]()

    nc.compile()
    return nc


def _get_nc():
    if "nc" not in _compiled:
        _compiled["nc"] = _build_bass()
    return _compiled["nc"]


def _ensure_ntff_hook():
    """Install the axon NTFF profile hook when the image's antenv lacks
    axon_hooks (trace support only; no-op when already present)."""
    import sys
    import types

    try:
        from antenv.axon_hooks import get_axon_ntff_profile_hook  # noqa: F401
        return
    except ImportError:
        pass
    try:
        import antenv
        from trn_agent_boot.trn_boot import _ntff_profile_via_ctypes

        so_path = "/opt/axon/libaxon_pjrt.so"
        if not os.path.exists(so_path):
            return
        hook = _ntff_profile_via_ctypes(so_path)
        mod = types.ModuleType("antenv.axon_hooks")
        state = {"hook": hook}
        mod.set_axon_ntff_profile_hook = lambda h: state.__setitem__("hook", h)
        mod.get_axon_ntff_profile_hook = lambda: state["hook"]
        sys.modules["antenv.axon_hooks"] = mod
        antenv.axon_hooks = mod
        # bucket upload is unavailable in this sandbox; keep artifacts local
        import concourse.bass_utils as _bu

        _bu.upload_artifacts = lambda tmpdir: tmpdir
    except Exception as e:  # pragma: no cover - best effort
        print(f"ntff hook install failed: {e}")


def kernel(**inputs):
    from concourse.bass_utils import run_bass_kernel_spmd

    nc = _get_nc()
    bf = ml_dtypes.bfloat16
    Q = np.asarray(inputs["Q"], dtype=np.float32)
    K = np.asarray(inputs["K"], dtype=np.float32)
    V = np.asarray(inputs["V"], dtype=np.float32)
    Wq = np.asarray(inputs["Wq"], dtype=np.float32)
    bq = np.asarray(inputs["bq"], dtype=np.float32)
    Wk = np.asarray(inputs["Wk"], dtype=np.float32)
    bk = np.asarray(inputs["bk"], dtype=np.float32)
    Wv = np.asarray(inputs["Wv"], dtype=np.float32)
    Wo = np.asarray(inputs["Wo"], dtype=np.float32)
    bv = np.asarray(inputs["bv"], dtype=np.float32)
    bo = np.asarray(inputs["bo"], dtype=np.float32)
    scale = np.float32(D_K ** -0.5)

    qt = np.ascontiguousarray(Q.T).astype(bf)
    kt = np.ascontiguousarray(K.T).astype(bf)
    vt = np.ascontiguousarray(V.T).astype(bf)

    def swz(w):  # (D_MODEL, DH) -> SBUF layout (128, KT*DH)
        return np.ascontiguousarray(
            w.reshape(KT, 128, DH).transpose(1, 0, 2).reshape(128, KT * DH)
        )

    in_maps = []
    for c in range(NCORES):
        h0 = HPC * c
        hs = list(range(h0, h0 + HPC))
        in_maps.append(
            dict(
                qt=qt,
                kt=kt,
                vt=vt,
                wq=swz(np.concatenate([Wq[h] for h in hs], axis=1)).astype(bf),
                wk=swz(np.concatenate([Wk[h] for h in hs], axis=1)).astype(bf),
                wv=swz(np.concatenate([Wv[h] for h in hs], axis=1)).astype(bf),
                wo=np.ascontiguousarray(Wo[h0 * D_V:(h0 + HPC) * D_V, :] * scale).astype(bf),
                bq=np.ascontiguousarray(bq[h0:h0 + HPC].reshape(DH, 1)),
                bk=np.ascontiguousarray(bk[h0:h0 + HPC].reshape(DH, 1)),
            )
        )

    trace = bool(int(os.environ.get("BASS_KERNEL_TRACE", "0")))
    if trace:
        _ensure_ntff_hook()
        tmpdir = os.environ.get("BASS_KERNEL_TMPDIR")
        res = run_bass_kernel_spmd(
            nc, in_maps, list(range(NCORES)), trace=True, tmpdir=tmpdir
        )
    else:
        res = run_bass_kernel_spmd(nc, in_maps, list(range(NCORES)))
    LAST_RESULT["exec_time_ns"] = res.exec_time_ns
    LAST_RESULT["res"] = res

    Y = np.zeros((N, D_MODEL), np.float32)
    for c in range(NCORES):
        Y += np.asarray(res.results[c]["out"], dtype=np.float32)
    Y += scale * (bv.reshape(-1) @ Wo) + bo
    return Y
